# revision 1
# baseline (speedup 1.0000x reference)
"""Trainium2 Bass kernel for DifferentiableTopK (Sinkhorn top-k masking).

Math (per batch row s in R^n, n=2048, K=256, eps=1e-3): the reference builds
log_P[i,j] = -(s_i - sorted(s)_j)^2/eps, runs 2 Sinkhorn normalizations
(col then row), and returns logsumexp over the first K (sorted) columns.

Kernel strategy (per batch, sorted domain, x = sorted scores descending):
  G[a,b] = exp(-1000*(x_a-x_b)^2) is symmetric, so all Sinkhorn reductions
  are weighted row sums = TensorEngine matvecs against stored G tiles:
    S1 = G @ 1 ; w1 = 1/S1 ; S2 = G @ w1 ; w2 = 1/S2 ; S3 = G @ w2
    w3 = 1/S3 ; S4 = G @ w3
    M[a] = 0 if a<K else -1000*(x_a - x_{K-1})^2
    ET[b,a] = exp(-1000*(x_a-x_b)^2 - M[a]) for b<K ; Ksum = ET^T @ w3[:K]
    out_sorted[a] = M[a] + log(Ksum[a] / S4[a])

  G/ET are built on the TensorEngine as an outer-product expansion of the
  squared distance: t0 = x_a*(2000 x_b) + (-1000 x_b^2) (+ (-M[a]) for ET),
  with every factor split into 3 bf16 limbs so a single-pass bf16 matmul
  (K=9 for G, K=12 for ET) reproduces fp32-level accuracy; one ScalarEngine
  Exp (bias = -1000 x_a^2, the natural_log_exp_and_others table) finishes
  each tile in bf16. All work is band-limited at 256-column granularity:
  entries with |x_a - x_b| > 0.26 contribute < e^-67 to any sum and are
  skipped (the host unions coverage over all rows so one SPMD program
  serves all 8 cores). S1 falls out of the Exp's accum_out, reduced and
  reciprocated per quarter so each Sinkhorn pass starts before its build
  fully finishes. Matvecs keep G stationary (128x128 bf16 blocks) so
  results land partition-major in PSUM — no transposes anywhere. The
  batch loop is software-pipelined; the device ships q = Ksum/S4 and the
  host applies out = M + ln(q) (keeps the Ln table set off the device).

Sharding: pure data parallel, 32 rows -> 8 cores x 4. Host does the sort and
tiny per-row prep; device does all n^2 work; host inverse-permutes.
"""
import math
import sys

sys.path.insert(0, "/opt/trn_rl_repo")

import numpy as np
import ml_dtypes
from contextlib import ExitStack

import concourse.bass as bass
import concourse.mybir as mybir
from concourse import bacc, tile
from concourse.bass_utils import run_bass_kernel_spmd

N = 2048
B = 32
NCORES = 8
BPC = B // NCORES
K = 256
NBLK = N // 128   # 16 partition blocks
NCH = N // 512    # 4 build chunks
BAND = 0.23       # build band: entries beyond are < e^-52, invisible in the sums
MVBAND = 0.19     # matvec band (subset of BAND): dropped pairs ~100x below bf16 noise
ETLIM = 52.0      # ET entries with exponent < -52 are invisible in the sums
F32 = mybir.dt.float32
BF16 = mybir.dt.bfloat16
AF = mybir.ActivationFunctionType
BF = ml_dtypes.bfloat16


def _coverage(xs_all):
    """Union (over all 32 rows) band coverage per batch slot.

    cov512[b][m]: build chunks (of 4) needed for G block m.
    cov128[b][m]: contraction 128-blocks k for the S matvecs.
    etch[b][blk]: build chunks needed for ET block blk (b<K rows).
    etmv[b][m]:   ET blocks blk contributing to Ksum output block m.
    """
    def runs(chunks):
        """Sorted 256-col chunk ids -> (start, n) runs of <=4 chunks
        (a 4x256-col psum tile is 2 banks)."""
        out = []
        for c in sorted(chunks):
            if out and out[-1][0] + out[-1][1] == c and out[-1][1] < 4:
                out[-1] = (out[-1][0], out[-1][1] + 1)
            else:
                out.append((c, 1))
        return out

    cov512 = [[set() for _ in range(NBLK)] for _ in range(BPC)]
    cov128 = [[set() for _ in range(NBLK)] for _ in range(BPC)]
    etch = [[set() for _ in range(2)] for _ in range(BPC)]
    etmv = [[set() for _ in range(NBLK)] for _ in range(BPC)]
    for row in range(B):
        b = row % BPC
        x = xs_all[row].astype(np.float64)
        M = np.where(np.arange(N) < K, 0.0, 1000.0 * (x - x[K - 1]) ** 2)
        bhi = [x[m * 128] for m in range(NBLK)]
        blo = [x[m * 128 + 127] for m in range(NBLK)]
        for m in range(NBLK):
            for kb in range(NBLK):
                if not (blo[m] - bhi[kb] > MVBAND or blo[kb] - bhi[m] > MVBAND):
                    cov128[b][m].add(kb)
            for c in range(2 * NCH):
                chi, clo = x[c * 256], x[c * 256 + 255]
                if not (blo[m] - chi > BAND or clo - bhi[m] > BAND):
                    cov512[b][m].add(c)
        # ET: entry (bb, a) alive iff 1000*(x_a-x_bb)^2 + M[a] <= ETLIM
        for blk in range(2):
            xb = x[blk * 128:(blk + 1) * 128]
            lo_b, hi_b = xb[-1], xb[0]
            # min over bb in block of (x_a - x_bb)^2 = interval distance.
            # ET exponent is -1000*gap^2 + M (M = +1000*(x_a - tau)^2
            # compensates the distance for far a), so alive needs
            # 1000*gap^2 - M <= ETLIM.
            gap = np.maximum(np.maximum(lo_b - x, x - hi_b), 0.0)
            alive = 1000.0 * gap * gap - M <= ETLIM
            for c in range(2 * NCH):
                if alive[c * 256:(c + 1) * 256].any():
                    etch[b][blk].add(c)
            for m in range(NBLK):
                if alive[m * 128:(m + 1) * 128].any():
                    etmv[b][m].add(blk)
    def span(chunks):
        c = sorted(chunks)
        return (c[0], c[-1] - c[0] + 1)  # fill holes: one contiguous run
    srt = lambda ll: [[sorted(s) for s in row] for row in ll]
    sp = lambda ll: [[span(s) for s in row] for row in ll]
    rr = lambda ll: [[runs(s) for s in row] for row in ll]
    return sp(cov512), srt(cov128), rr(etch), srt(etmv)


def build_program(cov512, cov128, etch, etmv):
    nc = bacc.Bacc("TRN2", target_bir_lowering=False, debug=False)

    d_lhs = nc.dram_tensor("lhsb", [BPC, 12, N], BF16, kind="ExternalInput").ap()
    d_rhs = nc.dram_tensor("rhsb", [BPC, 12, N], BF16, kind="ExternalInput").ap()
    d_eb = nc.dram_tensor("ebias", [BPC, 128, NBLK], F32, kind="ExternalInput").ap()
    d_out = nc.dram_tensor("out", [BPC, 128, NBLK], F32, kind="ExternalOutput").ap()

    with tile.TileContext(nc) as tc:
        with ExitStack() as ctx:
            gp = ctx.enter_context(tc.tile_pool(name="gpool", bufs=2 * NBLK))
            etp = ctx.enter_context(tc.tile_pool(name="etpool", bufs=4))
            rows = ctx.enter_context(tc.tile_pool(name="rows", bufs=3))
            tiny = ctx.enter_context(tc.tile_pool(name="tiny", bufs=4))
            acc = ctx.enter_context(tc.tile_pool(name="acc", bufs=3))
            fin = ctx.enter_context(tc.tile_pool(name="fin", bufs=BPC))
            pb = ctx.enter_context(tc.tile_pool(name="pbuild", bufs=3, space="PSUM"))
            pv = ctx.enter_context(tc.tile_pool(name="pvec", bufs=2, space="PSUM"))

            lhs0 = rows.tile([12, N], BF16, tag="lhsb")
            nc.sync.dma_start(lhs0[:], d_lhs[0])
            rhs0 = rows.tile([12, N], BF16, tag="rhsb")
            nc.sync.dma_start(rhs0[:], d_rhs[0])

            state = {}

            def emit_build(b, lhs0, rhs0):
                if b == 0:
                    lhsb, rhsb = lhs0, rhs0
                else:
                    lhsb = rows.tile([12, N], BF16, tag="lhsb")
                    nc.sync.dma_start(lhsb[:], d_lhs[b])
                    rhsb = rows.tile([12, N], BF16, tag="rhsb")
                    nc.sync.dma_start(rhsb[:], d_rhs[b])
                eb = tiny.tile([128, NBLK], F32, tag="eb")
                nc.sync.dma_start(eb[:], d_eb[b])

                s1acc = acc.tile([128, NBLK * 2], F32, tag="s1acc")
                nc.gpsimd.memset(s1acc[:], 0.0)
                gt = []
                for m in range(NBLK):
                    g = gp.tile([128, N], BF16, tag="g")
                    c0, ln = cov512[b][m]
                    pieces = [(p, min(4, ln - p)) for p in range(0, ln, 4)]
                    for ri, (p0, pl) in enumerate(pieces):
                        ps = pb.tile([128, pl * 256], F32, tag="pb")
                        for j in range(pl):
                            nc.tensor.matmul(
                                ps[:, j * 256:(j + 1) * 256],
                                lhsb[0:9, m * 128:(m + 1) * 128],
                                rhsb[0:9, (c0 + p0 + j) * 256:
                                     (c0 + p0 + j + 1) * 256],
                                start=True, stop=True)
                        nc.scalar.activation(
                            g[:, (c0 + p0) * 256:(c0 + p0 + pl) * 256], ps[:],
                            AF.Exp, bias=eb[:, m:m + 1], scale=1.0,
                            accum_out=s1acc[:, m * 2 + ri:m * 2 + ri + 1])
                    gt.append(g)

                et = []
                for blk in range(2):
                    e = etp.tile([128, N], BF16, tag="et")
                    for (c0, ln) in etch[b][blk]:
                        ps = pb.tile([128, ln * 256], F32, tag="pb")
                        for j in range(ln):
                            nc.tensor.matmul(
                                ps[:, j * 256:(j + 1) * 256],
                                lhsb[0:12, blk * 128:(blk + 1) * 128],
                                rhsb[0:12, (c0 + j) * 256:(c0 + j + 1) * 256],
                                start=True, stop=True)
                        nc.scalar.activation(e[:, c0 * 256:(c0 + ln) * 256], ps[:],
                                             AF.Exp, bias=eb[:, blk:blk + 1],
                                             scale=1.0)
                    et.append(e)
                state[b] = (gt, et, s1acc)

            def emit_chain(b):
                gt, et, s1acc = state.pop(b)
                # reduce S1 per quarter so the S2 pass starts as soon as
                # the first blocks' builds (and their accums) are done
                s1h = []
                for h in range(4):
                    sh = tiny.tile([128, 4], F32, tag="s")
                    nc.vector.tensor_reduce(
                        sh[:], s1acc[:, h * 8:(h + 1) * 8].rearrange(
                            "p (m c) -> p m c", c=2),
                        axis=mybir.AxisListType.X, op=mybir.AluOpType.add)
                    s1h.append((sh[:], h * 4, 4))

                def recip_cast(parts):
                    wb = tiny.tile([128, NBLK], BF16, tag="wb")
                    for ps, c0w, wd in parts:
                        wf = tiny.tile([128, wd], F32, tag="wf")
                        nc.vector.reciprocal(wf[:], ps)
                        nc.vector.tensor_copy(wb[:, c0w:c0w + wd], wf[:])
                    return wb

                def matvec(wb):
                    halves = []
                    for h in range(2):
                        ps = pv.tile([128, 8], F32, tag="pv")
                        for mi in range(8):
                            m = h * 8 + mi
                            ks = cov128[b][m]
                            for i, kb in enumerate(ks):
                                nc.tensor.matmul(
                                    ps[:, mi:mi + 1],
                                    gt[kb][:, m * 128:(m + 1) * 128],
                                    wb[:, kb:kb + 1],
                                    start=(i == 0), stop=(i == len(ks) - 1))
                        halves.append(ps)
                    return halves

                w1 = recip_cast(s1h)
                ps2h = matvec(w1)
                w2 = recip_cast([(ps2h[0][:], 0, 8), (ps2h[1][:], 8, 8)])
                ps3h = matvec(w2)
                w3 = recip_cast([(ps3h[0][:], 0, 8), (ps3h[1][:], 8, 8)])
                ps4h = matvec(w3)

                q = fin.tile([128, NBLK], F32, tag="q")
                for h in range(2):
                    hs = slice(h * 8, (h + 1) * 8)
                    pk = pv.tile([128, 8], F32, tag="pv")
                    for mi in range(8):
                        m = h * 8 + mi
                        bs = etmv[b][m]
                        for i, blk in enumerate(bs):
                            nc.tensor.matmul(pk[:, mi:mi + 1],
                                             et[blk][:, m * 128:(m + 1) * 128],
                                             w3[:, blk:blk + 1],
                                             start=(i == 0), stop=(i == len(bs) - 1))
                    r4 = tiny.tile([128, 8], F32, tag="r4")
                    nc.vector.reciprocal(r4[:], ps4h[h][:])
                    nc.vector.tensor_mul(q[:, hs], pk[:], r4[:])
                nc.sync.dma_start(d_out[b], q[:])

            for b in range(BPC):
                emit_build(b, lhs0, rhs0)
                if b >= 1:
                    # chains preempt builds whenever their deps are ready;
                    # build matmuls fill the ACT-paced stalls.
                    with tc.high_priority():
                        emit_chain(b - 1)
            with tc.high_priority():
                emit_chain(BPC - 1)

    nc.compile()
    return nc


_CACHE = {}


def _limbs3(v):
    """Split fp32 array into 3 bf16 limbs (exact to ~2^-27 relative)."""
    v = v.astype(np.float32)
    l0 = v.astype(BF)
    r = v - l0.astype(np.float32)
    l1 = r.astype(BF)
    l2 = (r - l1.astype(np.float32)).astype(BF)
    return l0, l1, l2


def prepare(scores: np.ndarray):
    """Host prep: sort, coverage, program build, per-core input maps."""
    scores = np.ascontiguousarray(np.asarray(scores, dtype=np.float32))
    assert scores.shape == (B, N), scores.shape

    orders = np.argsort(-scores, axis=-1, kind="stable")
    xs = np.take_along_axis(scores, orders, axis=-1)  # [B, N] sorted desc

    covs = _coverage(xs)
    key = (xs.tobytes(),)
    if key not in _CACHE:
        _CACHE.clear()
        _CACHE[key] = build_program(*covs)
    nc = _CACHE[key]

    d_tau = xs - xs[:, K - 1:K]
    M = np.where(np.arange(N)[None, :] < K, np.float32(0.0),
                 (np.float32(-1000.0) * d_tau * d_tau).astype(np.float32)
                 ).astype(np.float32)

    a0, a1, a2 = _limbs3(xs)
    c0, c1, c2 = _limbs3(np.float32(2000.0) * xs)
    dd0, dd1, dd2 = _limbs3(np.float32(-1000.0) * xs * xs)
    m0, m1, m2 = _limbs3(-M)
    one = np.ones_like(xs).astype(BF)
    # K rows pair lhs[k] with rhs[k]; products a_i*c_j kept for i+j<=2.
    lhsb = np.stack([a0, a0, a0, a1, a1, a2, one, one, one, one, one, one],
                    axis=1)  # [B, 12, N] bf16
    rhsb = np.stack([c0, c1, c2, c0, c1, c0, dd0, dd1, dd2, m0, m1, m2],
                    axis=1)
    ebias = (np.float32(-1000.0) * xs * xs).astype(np.float32)

    def pm(a):
        return np.ascontiguousarray(a.reshape(B, NBLK, 128).transpose(0, 2, 1))

    eb_pm = pm(ebias)
    in_maps = []
    for c in range(NCORES):
        sl = slice(c * BPC, (c + 1) * BPC)
        in_maps.append({
            "lhsb": np.ascontiguousarray(lhsb[sl]),
            "rhsb": np.ascontiguousarray(rhsb[sl]),
            "ebias": np.ascontiguousarray(eb_pm[sl]),
        })
    return nc, in_maps, orders, M


def postprocess(results, orders, M):
    out = np.empty((B, N), dtype=np.float32)
    for c in range(NCORES):
        o = results[c]["out"]  # [BPC, 128, NBLK] = q, sorted-domain
        for b in range(BPC):
            gb = c * BPC + b
            q = np.ascontiguousarray(o[b].T).reshape(N).astype(np.float64)
            out[gb, orders[gb]] = (M[gb].astype(np.float64) + np.log(q)
                                   ).astype(np.float32)
    return out


def kernel(scores: np.ndarray) -> np.ndarray:
    nc, in_maps, orders, M = prepare(scores)
    res = run_bass_kernel_spmd(nc, in_maps, core_ids=list(range(NCORES)))
    return postprocess(res.results, orders, M)


if __name__ == "__main__":
    x = np.random.randn(B, N).astype(np.float32)
    y = kernel(x)
    print("kernel ran, out shape", y.shape, "finite:", np.isfinite(y).all())



# revision 4
# speedup vs baseline: 1.2672x; 1.2672x over previous
"""Trainium2 Bass kernel for DifferentiableTopK (Sinkhorn top-k masking).

Math (per batch row s in R^n, n=2048, K=256, eps=1e-3): the reference builds
log_P[i,j] = -(s_i - sorted(s)_j)^2/eps, runs 2 Sinkhorn normalizations
(col then row), and returns logsumexp over the first K (sorted) columns.

Kernel strategy (per batch, sorted domain, x = sorted scores descending):
  G[a,b] = exp(-1000*(x_a-x_b)^2) is symmetric, so all Sinkhorn reductions
  are weighted row sums = TensorEngine matvecs against stored G tiles:
    S1 = G @ 1 ; w1 = 1/S1 ; S2 = G @ w1 ; w2 = 1/S2 ; S3 = G @ w2
    w3 = 1/S3 ; S4 = G @ w3
    Mp[a] = 0 if a<K else 1000*(x_a - x_{K-1})^2
    ET[b,a] = exp(-1000*(x_a-x_b)^2 + Mp[a]) for b<K ; Ksum = ET^T @ w3[:K]
    out_sorted[a] = -Mp[a] + log(Ksum[a] / S4[a])

  G/ET tiles are built bias-free on the TensorEngine as a 12-row bf16
  limb expansion (every fp32 factor split into 3 bf16 limbs, products
  kept to second order), so one ScalarEngine Exp with zero bias finishes
  a multi-block psum piece in a single instruction. All work is
  band-limited at 128-column granularity (entries beyond |dx|>0.09
  are < e^-8 relative and invisible at the 2e-2 gate); G is stored
  BANDED (only the alive span per 128-row block), so all 4 batch rows
  of a core stay resident in SBUF simultaneously. That allows the
  emission schedule to interleave the 4 rows' Sinkhorn chains with the
  later rows' builds: the PE never idles waiting for a reciprocal, which
  both hides chain latency and keeps the PE p-state clock at max.

Sharding: pure data parallel, 32 rows -> 8 cores x 4. Host does the sort and
tiny per-row prep; device does all n^2 work; host inverse-permutes.
"""
import sys

sys.path.insert(0, "/opt/trn_rl_repo")

import numpy as np
import ml_dtypes
from contextlib import ExitStack

import concourse.bass as bass
import concourse.mybir as mybir
from concourse import bacc, tile
from concourse.bass_utils import run_bass_kernel_spmd

N = 2048
B = 32
NCORES = 8
BPC = B // NCORES
K = 256
NBLK = N // 128   # 16 partition blocks == 16 column chunks (128-granular)
BAND = 0.09       # build band: dropped entries < e^-8 relative
MVBAND = 0.075    # matvec band
ETLIM = 8.0       # ET entries with exponent < -8 are dropped
PIECE = 1024      # max psum piece width (f32 cols) = 2 banks
F32 = mybir.dt.float32
BF16 = mybir.dt.bfloat16
AF = mybir.ActivationFunctionType
BF = ml_dtypes.bfloat16


def _coverage(xs_all):
    """Union (over the 8 cores' rows sharing a slot) band coverage.

    Returns per-slot:
      gsp[b][m] = (c0, c1) alive 128-col chunk span of G block m
      cov128[b][m] = contraction blocks kb for the S matvecs
      esp[b][blk] = (c0, c1) alive chunk span of ET block blk (b<K rows)
      etmv[b][m] = ET blocks contributing to Ksum output block m
    """
    gsp = [[set() for _ in range(NBLK)] for _ in range(BPC)]
    cov = [[set() for _ in range(NBLK)] for _ in range(BPC)]
    esp = [[set() for _ in range(2)] for _ in range(BPC)]
    emv = [[set() for _ in range(NBLK)] for _ in range(BPC)]
    for row in range(B):
        b = row % BPC
        x = xs_all[row].astype(np.float64)
        Mp = np.where(np.arange(N) < K, 0.0, 1000.0 * (x - x[K - 1]) ** 2)
        bhi = [x[m * 128] for m in range(NBLK)]
        blo = [x[m * 128 + 127] for m in range(NBLK)]
        for m in range(NBLK):
            for c in range(NBLK):
                if not (blo[m] - bhi[c] > BAND or blo[c] - bhi[m] > BAND):
                    gsp[b][m].add(c)
            for kb in range(NBLK):
                if not (blo[m] - bhi[kb] > MVBAND or blo[kb] - bhi[m] > MVBAND):
                    cov[b][m].add(kb)
        # ET: entry (bb, a) alive iff 1000*(x_a-x_bb)^2 - Mp[a] <= ETLIM
        for blk in range(2):
            xb = x[blk * 128:(blk + 1) * 128]
            gap = np.maximum(np.maximum(xb[-1] - x, x - xb[0]), 0.0)
            alive = 1000.0 * gap * gap - Mp <= ETLIM
            for c in range(NBLK):
                if alive[c * 128:(c + 1) * 128].any():
                    esp[b][blk].add(c)
            for m in range(NBLK):
                if alive[m * 128:(m + 1) * 128].any():
                    emv[b][m].add(blk)
    span = lambda s: (min(s), max(s)) if s else None
    gsp = [[span(s) for s in r] for r in gsp]
    esp = [[span(s) for s in r] for r in esp]
    cov = [[sorted(s) for s in r] for r in cov]
    emv = [[sorted(s) for s in r] for r in emv]
    return gsp, cov, esp, emv


def _pack_pieces(spans):
    """Pack consecutive blocks into psum pieces of <= PIECE f32 cols.

    spans: list of (blk_id, c0, c1). Returns list of pieces, each a list of
    (blk_id, c0, c1, col_off_in_piece)."""
    pieces, cur, w = [], [], 0
    for (mid, c0, c1) in spans:
        bw = (c1 - c0 + 1) * 128
        if cur and w + bw > PIECE:
            pieces.append(cur)
            cur, w = [], 0
        cur.append((mid, c0, c1, w))
        w += bw
    if cur:
        pieces.append(cur)
    return pieces


def build_program(gsp, cov128, esp, etmv):
    nc = bacc.Bacc("TRN2", target_bir_lowering=False, debug=False)

    d_lhs = nc.dram_tensor("lhsb", [BPC, 12, N], BF16, kind="ExternalInput").ap()
    d_rg = nc.dram_tensor("rhsg", [BPC, 12, N], BF16, kind="ExternalInput").ap()
    d_re = nc.dram_tensor("rhse", [BPC, 12, N], BF16, kind="ExternalInput").ap()
    d_out = nc.dram_tensor("out", [BPC, 128, NBLK], F32, kind="ExternalOutput").ap()

    # per-slot banded storage offsets
    goff = []   # goff[b][m] -> col offset of block m's band in g tile
    gW = []
    eoff = []
    eW = []
    for b in range(BPC):
        offs, o = [], 0
        for m in range(NBLK):
            offs.append(o)
            o += (gsp[b][m][1] - gsp[b][m][0] + 1) * 128
        goff.append(offs)
        gW.append(o)
        offs, o = [], 0
        for blk in range(2):
            offs.append(o)
            if esp[b][blk]:
                o += (esp[b][blk][1] - esp[b][blk][0] + 1) * 128
        eoff.append(offs)
        eW.append(o)

    with tile.TileContext(nc) as tc:
        with ExitStack() as ctx:
            # NOTE: tile-pool `bufs` is PER TAG.
            rows = ctx.enter_context(tc.tile_pool(name="rows", bufs=BPC))
            gp = ctx.enter_context(tc.tile_pool(name="gpool", bufs=BPC))
            etp = ctx.enter_context(tc.tile_pool(name="etpool", bufs=BPC))
            tiny = ctx.enter_context(tc.tile_pool(name="tiny", bufs=6))
            fin = ctx.enter_context(tc.tile_pool(name="fin", bufs=BPC))
            pb = ctx.enter_context(tc.tile_pool(name="pbuild", bufs=3, space="PSUM"))
            pv = ctx.enter_context(tc.tile_pool(name="pvec", bufs=2, space="PSUM"))

            # All input DMAs up front (everything fits in SBUF).
            lhs, rg, re = [], [], []
            for b in range(BPC):
                t = rows.tile([12, N], BF16, tag="lhs")
                nc.sync.dma_start(t[:], d_lhs[b])
                lhs.append(t)
                t = rows.tile([12, N], BF16, tag="rg")
                nc.sync.dma_start(t[:], d_rg[b])
                rg.append(t)
                t = rows.tile([12, N], BF16, tag="re")
                nc.sync.dma_start(t[:], d_re[b])
                re.append(t)
            ones = tiny.tile([128, NBLK], BF16, tag="ones")
            nc.gpsimd.memset(ones[:], 1.0)

            gt, et = {}, {}

            def build_units(b):
                """Emit-callables for slot b's G and ET builds."""
                g = gp.tile([128, gW[b]], BF16, tag="g")
                e = etp.tile([128, eW[b]], BF16, tag="et")
                gt[b], et[b] = g, e
                units = []
                gspans = [(m, gsp[b][m][0], gsp[b][m][1]) for m in range(NBLK)]
                for piece in _pack_pieces(gspans):
                    def gu(b=b, piece=piece, g=g):
                        w = sum((c1 - c0 + 1) * 128 for (_, c0, c1, _) in piece)
                        ps = pb.tile([128, w], F32, tag="pb")
                        for (m, c0, c1, po) in piece:
                            for j in range(c1 - c0 + 1):
                                nc.tensor.matmul(
                                    ps[:, po + j * 128:po + (j + 1) * 128],
                                    lhs[b][0:12, m * 128:(m + 1) * 128],
                                    rg[b][0:12, (c0 + j) * 128:(c0 + j + 1) * 128],
                                    start=True, stop=True)
                        o0 = goff[b][piece[0][0]]
                        nc.scalar.activation(g[:, o0:o0 + w], ps[:], AF.Exp)
                    units.append(gu)
                espans = [(blk, esp[b][blk][0], esp[b][blk][1])
                          for blk in range(2) if esp[b][blk]]
                # split wide ET spans into <= PIECE pieces
                esplit = []
                for (blk, c0, c1) in espans:
                    c = c0
                    while c <= c1:
                        c2 = min(c1, c + PIECE // 128 - 1)
                        esplit.append((blk, c, c2))
                        c = c2 + 1
                for (blk, c0, c1) in esplit:
                    def eu(b=b, blk=blk, c0=c0, c1=c1, e=e):
                        w = (c1 - c0 + 1) * 128
                        ps = pb.tile([128, w], F32, tag="pb")
                        for j in range(c1 - c0 + 1):
                            nc.tensor.matmul(
                                ps[:, j * 128:(j + 1) * 128],
                                lhs[b][0:12, blk * 128:(blk + 1) * 128],
                                re[b][0:12, (c0 + j) * 128:(c0 + j + 1) * 128],
                                start=True, stop=True)
                        o0 = eoff[b][blk] + (c0 - esp[b][blk][0]) * 128
                        nc.scalar.activation(e[:, o0:o0 + w], ps[:], AF.Exp)
                    units.append(eu)
                return units

            def s_pass(b, wsrc):
                ps = pv.tile([128, NBLK], F32, tag="pv")
                for m in range(NBLK):
                    ks = cov128[b][m]
                    for i, kb in enumerate(ks):
                        o = goff[b][kb] + (m - gsp[b][kb][0]) * 128
                        nc.tensor.matmul(ps[:, m:m + 1],
                                         gt[b][:, o:o + 128],
                                         wsrc[:, kb:kb + 1],
                                         start=(i == 0), stop=(i == len(ks) - 1))
                return ps

            def k_pass(b, wsrc):
                ps = pv.tile([128, NBLK], F32, tag="pv")
                for m in range(NBLK):
                    bs = etmv[b][m]
                    for i, blk in enumerate(bs):
                        o = eoff[b][blk] + (m - esp[b][blk][0]) * 128
                        nc.tensor.matmul(ps[:, m:m + 1],
                                         et[b][:, o:o + 128],
                                         wsrc[:, blk:blk + 1],
                                         start=(i == 0), stop=(i == len(bs) - 1))
                return ps

            def recip_cast(ps):
                sf = tiny.tile([128, NBLK], F32, tag="sf")
                nc.vector.reciprocal(sf[:], ps[:])
                wb = tiny.tile([128, NBLK], BF16, tag="wb")
                nc.vector.tensor_copy(wb[:], sf[:])
                return wb

            def chain_stages(b):
                st = {}

                def s1():
                    st["w1"] = recip_cast(s_pass(b, ones))

                def s2():
                    st["w2"] = recip_cast(s_pass(b, st.pop("w1")))

                def s3():
                    st["w3"] = recip_cast(s_pass(b, st.pop("w2")))

                def s4k():
                    w3 = st.pop("w3")
                    ps4 = s_pass(b, w3)
                    pk = k_pass(b, w3)
                    r4 = tiny.tile([128, NBLK], F32, tag="r4")
                    nc.vector.reciprocal(r4[:], ps4[:])
                    q = fin.tile([128, NBLK], F32, tag="q")
                    nc.vector.tensor_mul(q[:], pk[:], r4[:])
                    nc.sync.dma_start(d_out[b], q[:])
                return [s1, s2, s3, s4k]

            # ---- interleaved emission: builds row by row; ready chain
            # stages pumped round-robin between build units so the PE never
            # stalls on a reciprocal and the p-state clock stays up.
            ready = []      # [(b, stage_iter)]
            def pump(k):
                for _ in range(k):
                    if not ready:
                        return
                    b, it = ready.pop(0)
                    try:
                        stage = next(it)
                    except StopIteration:
                        continue
                    stage()
                    ready.append((b, it))

            for b in range(BPC):
                units = build_units(b)
                for i, u in enumerate(units):
                    u()
                    if i % 2 == 1:
                        pump(1)
                ready.append((b, iter(chain_stages(b))))
            pump(10 * BPC)

    nc.compile()
    return nc


_CACHE = {}


def _limbs3(v):
    """Split fp32 array into 3 bf16 limbs (exact to ~2^-27 relative)."""
    v = v.astype(np.float32)
    l0 = v.astype(BF)
    r = v - l0.astype(np.float32)
    l1 = r.astype(BF)
    l2 = (r - l1.astype(np.float32)).astype(BF)
    return l0, l1, l2


def prepare(scores: np.ndarray):
    """Host prep: sort, coverage, program build, per-core input maps."""
    scores = np.ascontiguousarray(np.asarray(scores, dtype=np.float32))
    assert scores.shape == (B, N), scores.shape

    orders = np.argsort(-scores, axis=-1, kind="stable")
    xs = np.take_along_axis(scores, orders, axis=-1)  # [B, N] sorted desc

    covs = _coverage(xs)
    key = (xs.tobytes(),)
    if key not in _CACHE:
        _CACHE.clear()
        _CACHE[key] = build_program(*covs)
    nc = _CACHE[key]

    d_tau = xs - xs[:, K - 1:K]
    Mp = np.where(np.arange(N)[None, :] < K, np.float32(0.0),
                  (np.float32(1000.0) * d_tau * d_tau).astype(np.float32)
                  ).astype(np.float32)

    a0, a1, a2 = _limbs3(xs)
    c0, c1, c2 = _limbs3(np.float32(2000.0) * xs)
    g0, g1, g2 = _limbs3(np.float32(-1000.0) * xs * xs)
    h0, h1, h2 = _limbs3(np.float32(-1000.0) * xs * xs + Mp)
    one = np.ones_like(xs).astype(BF)
    # 12 contraction rows: products a_i*c_j kept for i+j<=2, then the
    # per-partition bias (-1000 x_a^2 via lhs e-limbs x rhs ones) and the
    # per-column term (rhs g-limbs x lhs ones; h-limbs add +Mp for ET).
    lhsb = np.stack([a0, a0, a0, a1, a1, a2, g0, g1, g2, one, one, one],
                    axis=1)  # [B, 12, N] bf16
    rhsg = np.stack([c0, c1, c2, c0, c1, c0, one, one, one, g0, g1, g2],
                    axis=1)
    rhse = np.stack([c0, c1, c2, c0, c1, c0, one, one, one, h0, h1, h2],
                    axis=1)

    in_maps = []
    for c in range(NCORES):
        sl = slice(c * BPC, (c + 1) * BPC)
        in_maps.append({
            "lhsb": np.ascontiguousarray(lhsb[sl]),
            "rhsg": np.ascontiguousarray(rhsg[sl]),
            "rhse": np.ascontiguousarray(rhse[sl]),
        })
    return nc, in_maps, orders, Mp


def postprocess(results, orders, Mp):
    out = np.empty((B, N), dtype=np.float32)
    for c in range(NCORES):
        o = results[c]["out"]  # [BPC, 128, NBLK] = q, sorted-domain
        for b in range(BPC):
            gb = c * BPC + b
            q = np.ascontiguousarray(o[b].T).reshape(N).astype(np.float64)
            out[gb, orders[gb]] = (-Mp[gb].astype(np.float64) + np.log(q)
                                   ).astype(np.float32)
    return out


def kernel(scores: np.ndarray) -> np.ndarray:
    nc, in_maps, orders, Mp = prepare(scores)
    res = run_bass_kernel_spmd(nc, in_maps, core_ids=list(range(NCORES)))
    return postprocess(res.results, orders, Mp)


if __name__ == "__main__":
    x = np.random.randn(B, N).astype(np.float32)
    y = kernel(x)
    print("kernel ran, out shape", y.shape, "finite:", np.isfinite(y).all())


# revision 5
# speedup vs baseline: 1.5133x; 1.1942x over previous
"""Trainium2 Bass kernel for DifferentiableTopK (Sinkhorn top-k masking).

Math (per batch row s in R^n, n=2048, K=256, eps=1e-3): the reference builds
log_P[i,j] = -(s_i - sorted(s)_j)^2/eps, runs 2 Sinkhorn normalizations
(col then row), and returns logsumexp over the first K (sorted) columns.

Kernel strategy (per batch, sorted domain, x = sorted scores descending):
  G[a,b] = exp(-1000*(x_a-x_b)^2) is symmetric. The first Sinkhorn
  normalizer S1 = G @ 1 depends only on x, so the host computes it
  (banded f32 sum) and the device builds the column-scaled
    G1[a,b] = G[a,b] * w1_b,   w1 = 1/S1
  directly: ln w1 limbs ride as two extra contraction rows of the
  bias-free 16-row bf16 limb matmul, and one ScalarEngine Exp per
  multi-block psum piece finishes the tile. Then:
    S2 = rowsum(G1)                  (VectorEngine reduce of the stored band)
    v2 = 1/S2 ; T3 = mvT(v2)  = w1 * (G @ w2)      (PE matvec, transposed tile)
    v3 = 1/T3 ; u3 = w1 * v3  = w3 = 1/S3
    T4 = mvT(u3) = w1 * S4 ;  Ksum = ET1^T @ v3 = ET @ w3
    q = Ksum / T4 ;  out_sorted[a] = -Mp[a] + ln q_a + ln w1_a   (host)
  where Mp[a] = 0 for a<K else 1000*(x_a-x_{K-1})^2 and
  ET1[b,a] = exp(-1000*(x_a-x_b)^2 + Mp_a + ln w1_b) for b<K keeps the
  top-k column sums representable for far-below-threshold rows.

  All work is band-limited at 128-column granularity (dropped entries
  < e^-7 relative, invisible at the 2e-2 gate); G1 is stored BANDED so
  all 4 batch rows of a core stay resident in SBUF, and the emission
  schedule interleaves the rows' chains with later rows' builds: the PE
  never idles on a reciprocal and its p-state clock stays up.

Sharding: pure data parallel, 32 rows -> 8 cores x 4. Host does the sort and
O(n*bandwidth) prep; device does all n^2 work; host inverse-permutes.
"""
import sys

sys.path.insert(0, "/opt/trn_rl_repo")

import numpy as np
import ml_dtypes
from contextlib import ExitStack

import concourse.bass as bass
import concourse.mybir as mybir
from concourse import bacc, tile
from concourse.bass_utils import run_bass_kernel_spmd

N = 2048
B = 32
NCORES = 8
BPC = B // NCORES
K = 256
NBLK = N // 128   # 16 partition blocks == 16 column chunks (128-granular)
BAND = 0.085      # build band
MVBAND = 0.07     # matvec band
ETLIM = 7.0       # ET alive threshold
HB = 0.084        # host S1 band
PIECE = 1024      # max psum piece width (f32 cols) = 2 banks
F32 = mybir.dt.float32
BF16 = mybir.dt.bfloat16
AF = mybir.ActivationFunctionType
BF = ml_dtypes.bfloat16


def _coverage(xs_all):
    """Union (over the 8 cores' rows sharing a slot) band coverage."""
    gsp = [[set() for _ in range(NBLK)] for _ in range(BPC)]
    cov = [[set() for _ in range(NBLK)] for _ in range(BPC)]
    esp = [[set() for _ in range(2)] for _ in range(BPC)]
    emv = [[set() for _ in range(NBLK)] for _ in range(BPC)]
    for row in range(B):
        b = row % BPC
        x = xs_all[row].astype(np.float64)
        Mp = np.where(np.arange(N) < K, 0.0, 1000.0 * (x - x[K - 1]) ** 2)
        bhi = [x[m * 128] for m in range(NBLK)]
        blo = [x[m * 128 + 127] for m in range(NBLK)]
        for m in range(NBLK):
            for c in range(NBLK):
                if not (blo[m] - bhi[c] > BAND or blo[c] - bhi[m] > BAND):
                    gsp[b][m].add(c)
            for kb in range(NBLK):
                if not (blo[m] - bhi[kb] > MVBAND or blo[kb] - bhi[m] > MVBAND):
                    cov[b][m].add(kb)
        for blk in range(2):
            xb = x[blk * 128:(blk + 1) * 128]
            gap = np.maximum(np.maximum(xb[-1] - x, x - xb[0]), 0.0)
            alive = 1000.0 * gap * gap - Mp <= ETLIM
            for c in range(NBLK):
                if alive[c * 128:(c + 1) * 128].any():
                    esp[b][blk].add(c)
            for m in range(NBLK):
                if alive[m * 128:(m + 1) * 128].any():
                    emv[b][m].add(blk)
    span = lambda s: (min(s), max(s)) if s else None
    gsp = [[span(s) for s in r] for r in gsp]
    esp = [[span(s) for s in r] for r in esp]
    cov = [[sorted(s) for s in r] for r in cov]
    emv = [[sorted(s) for s in r] for r in emv]
    return gsp, cov, esp, emv


def _pack_pieces(spans):
    """Pack consecutive blocks into psum pieces of <= PIECE f32 cols."""
    pieces, cur, w = [], [], 0
    for (mid, c0, c1) in spans:
        bw = (c1 - c0 + 1) * 128
        if cur and w + bw > PIECE:
            pieces.append(cur)
            cur, w = [], 0
        cur.append((mid, c0, c1, w))
        w += bw
    if cur:
        pieces.append(cur)
    return pieces


def build_program(gsp, cov128, esp, etmv):
    nc = bacc.Bacc("TRN2", target_bir_lowering=False, debug=False)

    d_lhs = nc.dram_tensor("lhsb", [BPC, 16, N], BF16, kind="ExternalInput").ap()
    d_rg = nc.dram_tensor("rhsg", [BPC, 16, N], BF16, kind="ExternalInput").ap()
    d_re = nc.dram_tensor("rhse", [BPC, 16, N], BF16, kind="ExternalInput").ap()
    d_w1 = nc.dram_tensor("w1p", [BPC, 128, NBLK], F32, kind="ExternalInput").ap()
    d_out = nc.dram_tensor("out", [BPC, 128, NBLK], F32, kind="ExternalOutput").ap()

    goff, gW, eoff, eW = [], [], [], []
    for b in range(BPC):
        offs, o = [], 0
        for m in range(NBLK):
            offs.append(o)
            o += (gsp[b][m][1] - gsp[b][m][0] + 1) * 128
        goff.append(offs)
        gW.append(o)
        offs, o = [], 0
        for blk in range(2):
            offs.append(o)
            if esp[b][blk]:
                o += (esp[b][blk][1] - esp[b][blk][0] + 1) * 128
        eoff.append(offs)
        eW.append(o)

    with tile.TileContext(nc) as tc:
        with ExitStack() as ctx:
            # NOTE: tile-pool `bufs` is PER TAG.
            rows = ctx.enter_context(tc.tile_pool(name="rows", bufs=BPC))
            gp = ctx.enter_context(tc.tile_pool(name="gpool", bufs=BPC))
            etp = ctx.enter_context(tc.tile_pool(name="etpool", bufs=BPC))
            tiny = ctx.enter_context(tc.tile_pool(name="tiny", bufs=8))
            fin = ctx.enter_context(tc.tile_pool(name="fin", bufs=BPC))
            pb = ctx.enter_context(tc.tile_pool(name="pbuild", bufs=3, space="PSUM"))
            pv = ctx.enter_context(tc.tile_pool(name="pvec", bufs=2, space="PSUM"))

            lhs, rg, re, w1t = [], [], [], []
            for b in range(BPC):
                t = rows.tile([16, N], BF16, tag="lhs")
                nc.sync.dma_start(t[:], d_lhs[b])
                lhs.append(t)
                t = rows.tile([16, N], BF16, tag="rg")
                nc.sync.dma_start(t[:], d_rg[b])
                rg.append(t)
                t = rows.tile([16, N], BF16, tag="re")
                nc.sync.dma_start(t[:], d_re[b])
                re.append(t)
                t = tiny.tile([128, NBLK], F32, tag="w1")
                nc.sync.dma_start(t[:], d_w1[b])
                w1t.append(t)

            gt, et, s2t = {}, {}, {}

            def build_units(b):
                g = gp.tile([128, gW[b]], BF16, tag="g")
                e = etp.tile([128, eW[b]], BF16, tag="et")
                s2 = tiny.tile([128, NBLK], F32, tag="s2")
                gt[b], et[b], s2t[b] = g, e, s2
                units = []
                gspans = [(m, gsp[b][m][0], gsp[b][m][1]) for m in range(NBLK)]
                for piece in _pack_pieces(gspans):
                    def gu(b=b, piece=piece, g=g, s2=s2):
                        w = sum((c1 - c0 + 1) * 128 for (_, c0, c1, _) in piece)
                        ps = pb.tile([128, w], F32, tag="pb")
                        for (m, c0, c1, po) in piece:
                            for j in range(c1 - c0 + 1):
                                nc.tensor.matmul(
                                    ps[:, po + j * 128:po + (j + 1) * 128],
                                    lhs[b][0:16, m * 128:(m + 1) * 128],
                                    rg[b][0:16, (c0 + j) * 128:(c0 + j + 1) * 128],
                                    start=True, stop=True)
                        o0 = goff[b][piece[0][0]]
                        nc.scalar.activation(g[:, o0:o0 + w], ps[:], AF.Exp)
                        # S2 = rowsum(G1): per-block reduce of the fresh band
                        for (m, c0, c1, po) in piece:
                            bw = (c1 - c0 + 1) * 128
                            nc.vector.tensor_reduce(
                                s2[:, m:m + 1],
                                g[:, goff[b][m]:goff[b][m] + bw],
                                axis=mybir.AxisListType.X, op=mybir.AluOpType.add)
                    units.append(gu)
                espans = [(blk, esp[b][blk][0], esp[b][blk][1])
                          for blk in range(2) if esp[b][blk]]
                esplit = []
                for (blk, c0, c1) in espans:
                    c = c0
                    while c <= c1:
                        c2 = min(c1, c + PIECE // 128 - 1)
                        esplit.append((blk, c, c2))
                        c = c2 + 1
                for (blk, c0, c1) in esplit:
                    def eu(b=b, blk=blk, c0=c0, c1=c1, e=e):
                        w = (c1 - c0 + 1) * 128
                        ps = pb.tile([128, w], F32, tag="pb")
                        for j in range(c1 - c0 + 1):
                            nc.tensor.matmul(
                                ps[:, j * 128:(j + 1) * 128],
                                lhs[b][0:16, blk * 128:(blk + 1) * 128],
                                re[b][0:16, (c0 + j) * 128:(c0 + j + 1) * 128],
                                start=True, stop=True)
                        o0 = eoff[b][blk] + (c0 - esp[b][blk][0]) * 128
                        nc.scalar.activation(e[:, o0:o0 + w], ps[:], AF.Exp)
                    units.append(eu)
                return units

            def mv_pass(b, wsrc):
                ps = pv.tile([128, NBLK], F32, tag="pv")
                for m in range(NBLK):
                    ks = cov128[b][m]
                    for i, kb in enumerate(ks):
                        o = goff[b][kb] + (m - gsp[b][kb][0]) * 128
                        nc.tensor.matmul(ps[:, m:m + 1],
                                         gt[b][:, o:o + 128],
                                         wsrc[:, kb:kb + 1],
                                         start=(i == 0), stop=(i == len(ks) - 1))
                return ps

            def k_pass(b, wsrc):
                ps = pv.tile([128, NBLK], F32, tag="pv")
                for m in range(NBLK):
                    bs = etmv[b][m]
                    for i, blk in enumerate(bs):
                        o = eoff[b][blk] + (m - esp[b][blk][0]) * 128
                        nc.tensor.matmul(ps[:, m:m + 1],
                                         et[b][:, o:o + 128],
                                         wsrc[:, blk:blk + 1],
                                         start=(i == 0), stop=(i == len(bs) - 1))
                return ps

            def recip_bf(src):
                sf = tiny.tile([128, NBLK], F32, tag="sf")
                nc.vector.reciprocal(sf[:], src[:])
                wb = tiny.tile([128, NBLK], BF16, tag="wb")
                nc.vector.tensor_copy(wb[:], sf[:])
                return wb

            def chain_stages(b):
                st = {}

                def t3():
                    v2 = recip_bf(s2t[b])
                    ps3 = mv_pass(b, v2)
                    st["v3"] = recip_bf(ps3)
                    u3 = tiny.tile([128, NBLK], BF16, tag="wb")
                    nc.vector.tensor_mul(u3[:], w1t[b][:], st["v3"][:])
                    st["u3"] = u3

                def t4k():
                    ps4 = mv_pass(b, st.pop("u3"))
                    pk = k_pass(b, st.pop("v3"))
                    r4 = tiny.tile([128, NBLK], F32, tag="r4")
                    nc.vector.reciprocal(r4[:], ps4[:])
                    q = fin.tile([128, NBLK], F32, tag="q")
                    nc.vector.tensor_mul(q[:], pk[:], r4[:])
                    nc.sync.dma_start(d_out[b], q[:])
                return [t3, t4k]

            ready = []
            def pump(k):
                for _ in range(k):
                    if not ready:
                        return
                    b, it = ready.pop(0)
                    try:
                        stage = next(it)
                    except StopIteration:
                        continue
                    stage()
                    ready.append((b, it))

            for b in range(BPC):
                units = build_units(b)
                for i, u in enumerate(units):
                    u()
                    if i % 3 == 2:
                        pump(1)
                ready.append((b, iter(chain_stages(b))))
            pump(10 * BPC)

    nc.compile()
    return nc


_CACHE = {}


def _limbs(v, n):
    v = v.astype(np.float32)
    out = []
    for _ in range(n):
        l = v.astype(BF)
        out.append(l)
        v = v - l.astype(np.float32)
    return out


def _host_s1(x):
    """S1 = G @ 1 per row, banded f32 (input-only prep)."""
    lo = np.searchsorted(-x, -(x + HB), side="left")
    hi = np.searchsorted(-x, -(x - HB), side="right")
    Wm = int((hi - lo).max())
    ar = np.clip(lo[:, None] + np.arange(Wm)[None, :], 0, N - 1)
    mask = (lo[:, None] + np.arange(Wm)[None, :]) < hi[:, None]
    dx = x[:, None] - x[ar]
    return ((np.exp(-1000.0 * dx * dx).astype(np.float32) * mask)
            .sum(1).astype(np.float32))


def prepare(scores: np.ndarray):
    """Host prep: sort, S1, coverage, program build, per-core input maps."""
    scores = np.ascontiguousarray(np.asarray(scores, dtype=np.float32))
    assert scores.shape == (B, N), scores.shape

    orders = np.argsort(-scores, axis=-1, kind="stable")
    xs = np.take_along_axis(scores, orders, axis=-1)  # [B, N] sorted desc

    covs = _coverage(xs)
    key = (xs.tobytes(),)
    if key not in _CACHE:
        _CACHE.clear()
        _CACHE[key] = build_program(*covs)
    nc = _CACHE[key]

    d_tau = xs - xs[:, K - 1:K]
    Mp = np.where(np.arange(N)[None, :] < K, np.float32(0.0),
                  (np.float32(1000.0) * d_tau * d_tau).astype(np.float32)
                  ).astype(np.float32)

    S1 = np.stack([_host_s1(xs[r]) for r in range(B)])
    lnw1 = (-np.log(S1)).astype(np.float32)
    l0, l1 = _limbs(lnw1, 2)
    lnw1_eff = l0.astype(np.float64) + l1.astype(np.float64)
    w1_ship = np.exp(lnw1_eff).astype(np.float32)

    a0, a1, a2 = _limbs(xs, 3)
    c0, c1, c2 = _limbs(np.float32(2000.0) * xs, 3)
    g0, g1, g2 = _limbs(np.float32(-1000.0) * xs * xs, 3)
    h0, h1, h2 = _limbs(np.float32(-1000.0) * xs * xs + Mp, 3)
    one = np.ones_like(xs).astype(BF)
    zero = np.zeros_like(xs).astype(BF)
    # 16 contraction rows k (lhs_k * rhs_k):
    #  0-5: a_i x c_j (i+j<=2)   6-8: g-limbs x 1 (per-partition -1000x^2)
    #  9-11: 1 x g/h-limbs (per-col -1000x^2 [+Mp])
    #  12-13: 1 x lnw1-limbs (G col scaling) 14-15: lnw1-limbs x 1 (ET part scaling)
    lhsb = np.stack([a0, a0, a0, a1, a1, a2, g0, g1, g2, one, one, one,
                     one, one, l0, l1], axis=1)  # [B, 16, N]
    rhsg = np.stack([c0, c1, c2, c0, c1, c0, one, one, one, g0, g1, g2,
                     l0, l1, zero, zero], axis=1)
    rhse = np.stack([c0, c1, c2, c0, c1, c0, one, one, one, h0, h1, h2,
                     zero, zero, one, one], axis=1)

    w1_pm = np.ascontiguousarray(
        w1_ship.reshape(B, NBLK, 128).transpose(0, 2, 1))

    in_maps = []
    for c in range(NCORES):
        sl = slice(c * BPC, (c + 1) * BPC)
        in_maps.append({
            "lhsb": np.ascontiguousarray(lhsb[sl]),
            "rhsg": np.ascontiguousarray(rhsg[sl]),
            "rhse": np.ascontiguousarray(rhse[sl]),
            "w1p": np.ascontiguousarray(w1_pm[sl]),
        })
    return nc, in_maps, orders, Mp, lnw1_eff


def postprocess(results, orders, Mp, lnw1_eff):
    out = np.empty((B, N), dtype=np.float32)
    for c in range(NCORES):
        o = results[c]["out"]  # [BPC, 128, NBLK] = q, sorted-domain
        for b in range(BPC):
            gb = c * BPC + b
            q = np.ascontiguousarray(o[b].T).reshape(N).astype(np.float64)
            out[gb, orders[gb]] = (-Mp[gb].astype(np.float64) + np.log(q)
                                   + lnw1_eff[gb]).astype(np.float32)
    return out


def kernel(scores: np.ndarray) -> np.ndarray:
    nc, in_maps, orders, Mp, lnw1_eff = prepare(scores)
    res = run_bass_kernel_spmd(nc, in_maps, core_ids=list(range(NCORES)))
    return postprocess(res.results, orders, Mp, lnw1_eff)


if __name__ == "__main__":
    x = np.random.randn(B, N).astype(np.float32)
    y = kernel(x)
    print("kernel ran, out shape", y.shape, "finite:", np.isfinite(y).all())
